# revision 9
# baseline (speedup 1.0000x reference)
"""KMeans cross-attention layer on 8 TRN2 NeuronCores.

Strategy: shard point_features along N across 8 cores. The MLP head
(tiny, [128,256]) runs on host to produce mask_embeddings; each core
streams its point shard once: computes logits = P @ ME^T (f32, PE),
argmax via row-max + is_ge onehot, and a partial segment-sum
M += onehot^T @ P accumulated in PSUM. Host sums the 8 partials and
applies the bottleneck (LN -> Wb -> LN), again tiny.

HBM traffic per core = read 64MB (points) + write 32MB (logits), which
is the roofline for this memory-bound problem.
"""

import numpy as np
from contextlib import ExitStack

N_CORES = 8
Q = 128
C = 256
N_FULL = 500000
TILES_PER_BLOCK = 16           # 128-row tiles per DMA block
BLOCK_ROWS = 128 * TILES_PER_BLOCK   # 2048
N_BLOCKS = 31
ROWS_PER_CORE = BLOCK_ROWS * N_BLOCKS  # 63488  (8*63488 = 507904 >= 500000)
N_PAIRS = N_BLOCKS * TILES_PER_BLOCK // 2  # global pair count (2 tiles/pair)

EPS = 1e-5

_cache = {}


def _build_module():
    if "nc" in _cache:
        return _cache["nc"]
    import concourse.tile as tile
    from concourse import bacc, mybir
    from concourse.masks import make_identity

    fp32 = mybir.dt.float32
    fp32r = mybir.dt.float32r
    AX = mybir.AxisListType
    OP = mybir.AluOpType

    nc = bacc.Bacc(
        "TRN2", target_bir_lowering=False, debug=False, num_devices=N_CORES
    )
    pts = nc.dram_tensor("pts", [ROWS_PER_CORE, C], fp32, kind="ExternalInput").ap()
    meT = nc.dram_tensor("meT", [C, Q], fp32, kind="ExternalInput").ap()
    logits = nc.dram_tensor(
        "logits", [ROWS_PER_CORE, Q], fp32, kind="ExternalOutput"
    ).ap()
    pmem = nc.dram_tensor("pmem", [Q, C], fp32, kind="ExternalOutput").ap()

    with tile.TileContext(nc) as tc, ExitStack() as ctx:
        const = ctx.enter_context(tc.tile_pool(name="const", bufs=1))
        inpool = ctx.enter_context(tc.tile_pool(name="inblk", bufs=3))
        inr_pool = ctx.enter_context(tc.tile_pool(name="inr", bufs=4))
        outpool = ctx.enter_context(tc.tile_pool(name="outblk", bufs=3))
        ptT_pool = ctx.enter_context(tc.tile_pool(name="ptT", bufs=4))
        oh_pool = ctx.enter_context(tc.tile_pool(name="oh", bufs=6))
        rm_pool = ctx.enter_context(tc.tile_pool(name="rmax", bufs=6))
        misc = ctx.enter_context(tc.tile_pool(name="misc", bufs=1))
        ps_tr = ctx.enter_context(tc.tile_pool(name="ps_tr", bufs=3, space="PSUM"))
        ps_log = ctx.enter_context(tc.tile_pool(name="ps_log", bufs=3, space="PSUM"))
        ps_mem = ctx.enter_context(tc.tile_pool(name="ps_mem", bufs=1, space="PSUM"))

        ident = const.tile([128, 128], fp32)
        make_identity(nc, ident[:])
        # meT DRAM [256,128]; chunk k (rows k*128..) -> me_sb[:, k, :]
        me_sb = const.tile([128, 2, Q], fp32)
        nc.sync.dma_start(me_sb[:], meT.rearrange("(k c) q -> c k q", c=128))

        mem_acc = ps_mem.tile([Q, C], fp32)  # one bank, accumulates all blocks

        # per-pair state carried across the software pipeline
        inblk_of = {}   # block -> tile
        outblk_of = {}  # block -> tile
        ptT_of = {}     # pair g -> sbuf tile [128, 512]
        oh_of = {}      # pair g -> onehot tile [128, 2*Q]
        inr_of = {}     # pair g -> fp32r point rows [128, 512]
        scat_count = 0

        def stage_A(g):
            """DMA block if needed; transpose pair g's two tiles; copy to SBUF."""
            b, pair = divmod(g, TILES_PER_BLOCK // 2)
            if pair == 0:
                inblk = inpool.tile([128, TILES_PER_BLOCK * C], fp32, name="inblk")
                nc.sync.dma_start(
                    inblk[:].rearrange("p (j c) -> p j c", j=TILES_PER_BLOCK),
                    pts[b * BLOCK_ROWS : (b + 1) * BLOCK_ROWS, :].rearrange(
                        "(p j) c -> p j c", p=128
                    ),
                )
                inblk_of[b] = inblk
                outblk_of[b] = outpool.tile(
                    [128, TILES_PER_BLOCK * Q], fp32, name="outblk"
                )
            inblk = inblk_of[b]
            j0 = pair * 2
            trb = ps_tr.tile([128, 512], fp32)
            for jj in range(2):
                for k in range(2):
                    nc.tensor.transpose(
                        trb[:, (jj * 2 + k) * 128 : (jj * 2 + k + 1) * 128],
                        inblk[:, (j0 + jj) * C + k * 128 : (j0 + jj) * C + (k + 1) * 128],
                        ident[:],
                    )
            ptT = ptT_pool.tile([128, 512], fp32)
            nc.vector.tensor_copy(ptT[:, 0:256], trb[:, 0:256])
            nc.scalar.copy(ptT[:, 256:512], trb[:, 256:512])
            ptT_of[g] = ptT
            # fp32r-rounded copy of the pair's point rows for the scatter
            # matmul (runs at full PE rate; rounding costs ~2^-13 on P)
            inr = inr_pool.tile([128, 512], fp32r, name="inr")
            nc.gpsimd.tensor_copy(inr[:], inblk[:, j0 * C : (j0 + 2) * C])
            inr_of[g] = inr

        def stage_B(g):
            """Logits matmuls for pair g; stage to outblk; rowmax; onehot."""
            b, pair = divmod(g, TILES_PER_BLOCK // 2)
            j0 = pair * 2
            ptT = ptT_of.pop(g)
            outblk = outblk_of[b]
            logb = ps_log.tile([128, 2 * Q], fp32)
            for jj in range(2):
                nc.tensor.matmul(
                    logb[:, jj * Q : (jj + 1) * Q],
                    ptT[:, jj * 256 : jj * 256 + 128],
                    me_sb[:, 0, :],
                    start=True,
                    stop=False,
                )
                nc.tensor.matmul(
                    logb[:, jj * Q : (jj + 1) * Q],
                    ptT[:, jj * 256 + 128 : jj * 256 + 256],
                    me_sb[:, 1, :],
                    start=False,
                    stop=True,
                )
            nc.scalar.copy(outblk[:, j0 * Q : (j0 + 2) * Q], logb[:])
            rmax = rm_pool.tile([128, 2], fp32)
            nc.vector.tensor_reduce(
                rmax[:],
                outblk[:, j0 * Q : (j0 + 2) * Q].rearrange("p (j q) -> p j q", j=2),
                axis=AX.X,
                op=OP.max,
            )
            oh = oh_pool.tile([128, 2 * Q], fp32r)
            for jj in range(2):
                nc.vector.tensor_scalar(
                    oh[:, jj * Q : (jj + 1) * Q],
                    outblk[:, (j0 + jj) * Q : (j0 + jj + 1) * Q],
                    rmax[:, jj : jj + 1],
                    None,
                    op0=OP.is_ge,
                )
            oh_of[g] = oh

        def stage_C(g):
            """Scatter matmuls for pair g into the persistent accumulator."""
            nonlocal scat_count
            b, pair = divmod(g, TILES_PER_BLOCK // 2)
            j0 = pair * 2
            oh = oh_of.pop(g)
            inr = inr_of.pop(g)
            for jj in range(2):
                nc.tensor.matmul(
                    mem_acc[:],
                    oh[:, jj * Q : (jj + 1) * Q],
                    inr[:, jj * C : (jj + 1) * C],
                    start=(scat_count == 0),
                    stop=(scat_count == 2 * N_PAIRS - 1),
                    skip_group_check=True,
                )
                scat_count += 1

        def flush_block_out(b):
            outblk = outblk_of.pop(b)
            nc.sync.dma_start(
                logits[b * BLOCK_ROWS : (b + 1) * BLOCK_ROWS, :].rearrange(
                    "(p j) q -> p j q", p=128
                ),
                outblk[:].rearrange("p (j q) -> p j q", j=TILES_PER_BLOCK),
            )

        PAIRS_PER_BLOCK = TILES_PER_BLOCK // 2
        for g in range(N_PAIRS + 2):
            if g < N_PAIRS:
                stage_A(g)
            if 1 <= g <= N_PAIRS:
                stage_B(g - 1)
                if (g - 1) % PAIRS_PER_BLOCK == PAIRS_PER_BLOCK - 1:
                    flush_block_out((g - 1) // PAIRS_PER_BLOCK)
            if g >= 2:
                stage_C(g - 2)
            # drop inblk refs for fully-consumed blocks
            done_b = (g - 2) // PAIRS_PER_BLOCK if g >= 2 else -1
            if done_b >= 0 and (g - 2) % PAIRS_PER_BLOCK == PAIRS_PER_BLOCK - 1:
                inblk_of.pop(done_b, None)

        pm_sb = misc.tile([Q, C], fp32)
        nc.vector.tensor_copy(pm_sb[:], mem_acc[:])
        nc.sync.dma_start(pmem[:], pm_sb[:])

    nc.compile()
    _cache["nc"] = nc
    return nc


def _layer_norm(x, w, b):
    m = x.mean(axis=-1, keepdims=True, dtype=np.float32)
    v = np.mean((x - m) ** 2, axis=-1, keepdims=True, dtype=np.float32)
    return ((x - m) / np.sqrt(v + EPS) * w + b).astype(np.float32)


def _host_head(cluster_centers, ln0_w, ln0_b, W1, b1, W2, b2, W3, b3):
    x = _layer_norm(cluster_centers, ln0_w, ln0_b)
    x = np.maximum(x @ W1.T + b1, 0.0).astype(np.float32)
    x = np.maximum(x @ W2.T + b2, 0.0).astype(np.float32)
    return (x @ W3.T + b3).astype(np.float32)  # mask_embeddings [Q, C]


def _make_in_maps(point_features, me):
    meT = np.ascontiguousarray(me.T).astype(np.float32)  # [C, Q]
    padded = np.zeros((N_CORES * ROWS_PER_CORE, C), np.float32)
    padded[:N_FULL] = point_features
    shards = padded.reshape(N_CORES, ROWS_PER_CORE, C)
    return [
        {"pts": np.ascontiguousarray(shards[i]), "meT": meT} for i in range(N_CORES)
    ]


def run_device(in_maps, trace=False, tmpdir=None):
    from concourse import bass_utils

    nc = _build_module()
    return bass_utils.run_bass_kernel_spmd(
        nc,
        in_maps,
        core_ids=list(range(N_CORES)),
        trace=trace,
        tmpdir=tmpdir,
    )


def kernel(
    cluster_centers,
    point_features,
    ln0_w,
    ln0_b,
    W1,
    b1,
    W2,
    b2,
    W3,
    b3,
    lnb1_w,
    lnb1_b,
    Wb,
    lnb2_w,
    lnb2_b,
):
    cluster_centers = np.asarray(cluster_centers, np.float32)
    point_features = np.asarray(point_features, np.float32)
    args = [ln0_w, ln0_b, W1, b1, W2, b2, W3, b3]
    ln0_w, ln0_b, W1, b1, W2, b2, W3, b3 = (np.asarray(a, np.float32) for a in args)
    lnb1_w = np.asarray(lnb1_w, np.float32)
    lnb1_b = np.asarray(lnb1_b, np.float32)
    Wb = np.asarray(Wb, np.float32)
    lnb2_w = np.asarray(lnb2_w, np.float32)
    lnb2_b = np.asarray(lnb2_b, np.float32)

    me = _host_head(cluster_centers, ln0_w, ln0_b, W1, b1, W2, b2, W3, b3)
    in_maps = _make_in_maps(point_features, me)
    res = run_device(in_maps)

    pred_logits = np.concatenate(
        [res.results[i]["logits"] for i in range(N_CORES)], axis=0
    )[:N_FULL]
    cluster_memory = np.zeros((Q, C), np.float64)
    for i in range(N_CORES):
        cluster_memory += res.results[i]["pmem"].astype(np.float64)
    cluster_memory = cluster_memory.astype(np.float32)

    y = _layer_norm(cluster_memory, lnb1_w, lnb1_b)
    y = (y @ Wb.T).astype(np.float32)
    y = _layer_norm(y, lnb2_w, lnb2_b)
    return pred_logits.astype(np.float32), (cluster_centers + y).astype(np.float32)


# revision 12
# speedup vs baseline: 1.5988x; 1.5988x over previous
"""KMeans cross-attention layer on 8 TRN2 NeuronCores.

Strategy: shard point_features along N across 8 cores. The MLP head
(tiny, [128,256]) runs on host to produce mask_embeddings; each core
streams its point shard once: computes logits = P @ ME^T (f32, PE),
argmax via row-max + is_ge onehot, and a partial segment-sum
M += onehot^T @ P accumulated in PSUM. Host sums the 8 partials and
applies the bottleneck (LN -> Wb -> LN), again tiny.

HBM traffic per core = read 64MB (points) + write 32MB (logits), which
is the roofline for this memory-bound problem.
"""

import numpy as np
from contextlib import ExitStack

N_CORES = 8
Q = 128
C = 256
N_FULL = 500000
TILES_PER_BLOCK = 16           # 128-row tiles per DMA block
BLOCK_ROWS = 128 * TILES_PER_BLOCK   # 2048
N_BLOCKS = 31
ROWS_PER_CORE = BLOCK_ROWS * N_BLOCKS  # 63488  (8*63488 = 507904 >= 500000)
N_PAIRS = N_BLOCKS * TILES_PER_BLOCK // 2  # global pair count (2 tiles/pair)

EPS = 1e-5

_cache = {}


def _build_module():
    if "nc" in _cache:
        return _cache["nc"]
    import concourse.tile as tile
    from concourse import bacc, mybir
    from concourse.masks import make_identity

    fp32 = mybir.dt.float32
    bf16 = mybir.dt.bfloat16
    AX = mybir.AxisListType
    OP = mybir.AluOpType

    nc = bacc.Bacc(
        "TRN2", target_bir_lowering=False, debug=False, num_devices=N_CORES
    )
    pts = nc.dram_tensor("pts", [ROWS_PER_CORE, C], fp32, kind="ExternalInput").ap()
    meT = nc.dram_tensor("meT", [C, Q], fp32, kind="ExternalInput").ap()
    logits = nc.dram_tensor(
        "logits", [ROWS_PER_CORE, Q], fp32, kind="ExternalOutput"
    ).ap()
    pmem = nc.dram_tensor("pmem", [Q, C], fp32, kind="ExternalOutput").ap()

    with tile.TileContext(nc) as tc, ExitStack() as ctx:
        const = ctx.enter_context(tc.tile_pool(name="const", bufs=1))
        inpool = ctx.enter_context(tc.tile_pool(name="inblk", bufs=3))
        inr_pool = ctx.enter_context(tc.tile_pool(name="inr", bufs=4))
        outpool = ctx.enter_context(tc.tile_pool(name="outblk", bufs=3))
        ptT_pool = ctx.enter_context(tc.tile_pool(name="ptT", bufs=4))
        oh_pool = ctx.enter_context(tc.tile_pool(name="oh", bufs=6))
        rm_pool = ctx.enter_context(tc.tile_pool(name="rmax", bufs=6))
        misc = ctx.enter_context(tc.tile_pool(name="misc", bufs=1))
        ps_tr = ctx.enter_context(tc.tile_pool(name="ps_tr", bufs=3, space="PSUM"))
        ps_log = ctx.enter_context(tc.tile_pool(name="ps_log", bufs=3, space="PSUM"))
        ps_mem = ctx.enter_context(tc.tile_pool(name="ps_mem", bufs=1, space="PSUM"))

        ident = const.tile([128, 128], fp32)
        make_identity(nc, ident[:])
        # meT DRAM [256,128]; chunk k (rows k*128..) -> me_sb[:, k, :]
        me_sb = const.tile([128, 2, Q], fp32)
        nc.sync.dma_start(me_sb[:], meT.rearrange("(k c) q -> c k q", c=128))

        mem_acc = ps_mem.tile([Q, C], fp32)  # one bank, accumulates all blocks

        # per-pair state carried across the software pipeline
        inblk_of = {}   # block -> tile
        outblk_of = {}  # block -> tile
        ptT_of = {}     # pair g -> sbuf tile [128, 512]
        oh_of = {}      # pair g -> onehot tile [128, 2*Q]
        inr_of = {}     # pair g -> fp32r point rows [128, 512]
        scat_count = 0

        def stage_A(g):
            """DMA block if needed; transpose pair g's two tiles; copy to SBUF."""
            b, pair = divmod(g, TILES_PER_BLOCK // 2)
            if pair == 0:
                inblk = inpool.tile([128, TILES_PER_BLOCK * C], fp32, name="inblk")
                nc.sync.dma_start(
                    inblk[:].rearrange("p (j c) -> p j c", j=TILES_PER_BLOCK),
                    pts[b * BLOCK_ROWS : (b + 1) * BLOCK_ROWS, :].rearrange(
                        "(p j) c -> p j c", p=128
                    ),
                )
                inblk_of[b] = inblk
                outblk_of[b] = outpool.tile(
                    [128, TILES_PER_BLOCK * Q], fp32, name="outblk"
                )
            inblk = inblk_of[b]
            j0 = pair * 2
            trb = ps_tr.tile([128, 512], fp32)
            for jj in range(2):
                for k in range(2):
                    nc.tensor.transpose(
                        trb[:, (jj * 2 + k) * 128 : (jj * 2 + k + 1) * 128],
                        inblk[:, (j0 + jj) * C + k * 128 : (j0 + jj) * C + (k + 1) * 128],
                        ident[:],
                    )
            ptT = ptT_pool.tile([128, 512], fp32)
            nc.vector.tensor_copy(ptT[:, 0:256], trb[:, 0:256])
            nc.scalar.copy(ptT[:, 256:512], trb[:, 256:512])
            ptT_of[g] = ptT
            # bf16 copy of the pair's point rows for the scatter matmul
            # (runs at full PE rate; bf16 rounding of P costs ~1e-3 on the
            # cluster sums, far under tolerance; onehot is exact in bf16)
            inr = inr_pool.tile([128, 512], bf16, name="inr")
            nc.scalar.copy(inr[:], inblk[:, j0 * C : (j0 + 2) * C])
            inr_of[g] = inr

        def stage_B(g):
            """Logits matmuls for pair g; stage to outblk; rowmax; onehot."""
            b, pair = divmod(g, TILES_PER_BLOCK // 2)
            j0 = pair * 2
            ptT = ptT_of.pop(g)
            outblk = outblk_of[b]
            logb = ps_log.tile([128, 2 * Q], fp32)
            for jj in range(2):
                nc.tensor.matmul(
                    logb[:, jj * Q : (jj + 1) * Q],
                    ptT[:, jj * 256 : jj * 256 + 128],
                    me_sb[:, 0, :],
                    start=True,
                    stop=False,
                )
                nc.tensor.matmul(
                    logb[:, jj * Q : (jj + 1) * Q],
                    ptT[:, jj * 256 + 128 : jj * 256 + 256],
                    me_sb[:, 1, :],
                    start=False,
                    stop=True,
                )
            nc.scalar.copy(outblk[:, j0 * Q : (j0 + 2) * Q], logb[:])
            rmax = rm_pool.tile([128, 2], fp32)
            nc.vector.tensor_reduce(
                rmax[:],
                outblk[:, j0 * Q : (j0 + 2) * Q].rearrange("p (j q) -> p j q", j=2),
                axis=AX.X,
                op=OP.max,
            )
            oh = oh_pool.tile([128, 2 * Q], bf16)
            for jj in range(2):
                nc.vector.tensor_scalar(
                    oh[:, jj * Q : (jj + 1) * Q],
                    outblk[:, (j0 + jj) * Q : (j0 + jj + 1) * Q],
                    rmax[:, jj : jj + 1],
                    None,
                    op0=OP.is_ge,
                )
            oh_of[g] = oh

        def stage_C(g):
            """Scatter matmuls for pair g into the persistent accumulator."""
            nonlocal scat_count
            b, pair = divmod(g, TILES_PER_BLOCK // 2)
            j0 = pair * 2
            oh = oh_of.pop(g)
            inr = inr_of.pop(g)
            for jj in range(2):
                nc.tensor.matmul(
                    mem_acc[:],
                    oh[:, jj * Q : (jj + 1) * Q],
                    inr[:, jj * C : (jj + 1) * C],
                    start=(scat_count == 0),
                    stop=(scat_count == 2 * N_PAIRS - 1),
                    skip_group_check=True,
                )
                scat_count += 1

        def flush_block_out(b):
            outblk = outblk_of.pop(b)
            nc.sync.dma_start(
                logits[b * BLOCK_ROWS : (b + 1) * BLOCK_ROWS, :].rearrange(
                    "(p j) q -> p j q", p=128
                ),
                outblk[:].rearrange("p (j q) -> p j q", j=TILES_PER_BLOCK),
            )

        PAIRS_PER_BLOCK = TILES_PER_BLOCK // 2
        for g in range(N_PAIRS + 2):
            if g < N_PAIRS:
                stage_A(g)
            if 1 <= g <= N_PAIRS:
                stage_B(g - 1)
                if (g - 1) % PAIRS_PER_BLOCK == PAIRS_PER_BLOCK - 1:
                    flush_block_out((g - 1) // PAIRS_PER_BLOCK)
            if g >= 2:
                stage_C(g - 2)
            # drop inblk refs for fully-consumed blocks
            done_b = (g - 2) // PAIRS_PER_BLOCK if g >= 2 else -1
            if done_b >= 0 and (g - 2) % PAIRS_PER_BLOCK == PAIRS_PER_BLOCK - 1:
                inblk_of.pop(done_b, None)

        pm_sb = misc.tile([Q, C], fp32)
        nc.vector.tensor_copy(pm_sb[:], mem_acc[:])
        nc.sync.dma_start(pmem[:], pm_sb[:])

    nc.compile()
    _cache["nc"] = nc
    return nc


def _layer_norm(x, w, b):
    m = x.mean(axis=-1, keepdims=True, dtype=np.float32)
    v = np.mean((x - m) ** 2, axis=-1, keepdims=True, dtype=np.float32)
    return ((x - m) / np.sqrt(v + EPS) * w + b).astype(np.float32)


def _host_head(cluster_centers, ln0_w, ln0_b, W1, b1, W2, b2, W3, b3):
    x = _layer_norm(cluster_centers, ln0_w, ln0_b)
    x = np.maximum(x @ W1.T + b1, 0.0).astype(np.float32)
    x = np.maximum(x @ W2.T + b2, 0.0).astype(np.float32)
    return (x @ W3.T + b3).astype(np.float32)  # mask_embeddings [Q, C]


def _make_in_maps(point_features, me):
    meT = np.ascontiguousarray(me.T).astype(np.float32)  # [C, Q]
    padded = np.zeros((N_CORES * ROWS_PER_CORE, C), np.float32)
    padded[:N_FULL] = point_features
    shards = padded.reshape(N_CORES, ROWS_PER_CORE, C)
    return [
        {"pts": np.ascontiguousarray(shards[i]), "meT": meT} for i in range(N_CORES)
    ]


def run_device(in_maps, trace=False, tmpdir=None):
    from concourse import bass_utils

    nc = _build_module()
    return bass_utils.run_bass_kernel_spmd(
        nc,
        in_maps,
        core_ids=list(range(N_CORES)),
        trace=trace,
        tmpdir=tmpdir,
    )


def kernel(
    cluster_centers,
    point_features,
    ln0_w,
    ln0_b,
    W1,
    b1,
    W2,
    b2,
    W3,
    b3,
    lnb1_w,
    lnb1_b,
    Wb,
    lnb2_w,
    lnb2_b,
):
    cluster_centers = np.asarray(cluster_centers, np.float32)
    point_features = np.asarray(point_features, np.float32)
    args = [ln0_w, ln0_b, W1, b1, W2, b2, W3, b3]
    ln0_w, ln0_b, W1, b1, W2, b2, W3, b3 = (np.asarray(a, np.float32) for a in args)
    lnb1_w = np.asarray(lnb1_w, np.float32)
    lnb1_b = np.asarray(lnb1_b, np.float32)
    Wb = np.asarray(Wb, np.float32)
    lnb2_w = np.asarray(lnb2_w, np.float32)
    lnb2_b = np.asarray(lnb2_b, np.float32)

    me = _host_head(cluster_centers, ln0_w, ln0_b, W1, b1, W2, b2, W3, b3)
    in_maps = _make_in_maps(point_features, me)
    res = run_device(in_maps)

    pred_logits = np.concatenate(
        [res.results[i]["logits"] for i in range(N_CORES)], axis=0
    )[:N_FULL]
    cluster_memory = np.zeros((Q, C), np.float64)
    for i in range(N_CORES):
        cluster_memory += res.results[i]["pmem"].astype(np.float64)
    cluster_memory = cluster_memory.astype(np.float32)

    y = _layer_norm(cluster_memory, lnb1_w, lnb1_b)
    y = (y @ Wb.T).astype(np.float32)
    y = _layer_norm(y, lnb2_w, lnb2_b)
    return pred_logits.astype(np.float32), (cluster_centers + y).astype(np.float32)


# revision 16
# speedup vs baseline: 1.6041x; 1.0033x over previous
"""KMeans cross-attention layer on 8 TRN2 NeuronCores.

Strategy: shard point_features along N across 8 cores. The MLP head
(tiny, [128,256]) runs on host to produce mask_embeddings; each core
streams its point shard once: computes logits = P @ ME^T (f32, PE),
argmax via row-max + is_ge onehot, and a partial segment-sum
M += onehot^T @ P accumulated in PSUM. Host sums the 8 partials and
applies the bottleneck (LN -> Wb -> LN), again tiny.

HBM traffic per core = read 64MB (points) + write 32MB (logits), which
is the roofline for this memory-bound problem.
"""

import numpy as np
from contextlib import ExitStack

N_CORES = 8
Q = 128
C = 256
N_FULL = 500000
TILES_PER_BLOCK = 16           # 128-row tiles per DMA block
BLOCK_ROWS = 128 * TILES_PER_BLOCK   # 2048
N_BLOCKS = 31
ROWS_PER_CORE = BLOCK_ROWS * N_BLOCKS  # 63488  (8*63488 = 507904 >= 500000)
N_PAIRS = N_BLOCKS * TILES_PER_BLOCK // 2  # global pair count (2 tiles/pair)

EPS = 1e-5

_cache = {}


def _build_module():
    if "nc" in _cache:
        return _cache["nc"]
    import concourse.tile as tile
    from concourse import bacc, mybir
    from concourse.masks import make_identity

    fp32 = mybir.dt.float32
    bf16 = mybir.dt.bfloat16
    AX = mybir.AxisListType
    OP = mybir.AluOpType

    nc = bacc.Bacc(
        "TRN2", target_bir_lowering=False, debug=False, num_devices=N_CORES
    )
    pts = nc.dram_tensor("pts", [ROWS_PER_CORE, C], fp32, kind="ExternalInput").ap()
    meT = nc.dram_tensor("meT", [C, Q], fp32, kind="ExternalInput").ap()
    logits = nc.dram_tensor(
        "logits", [ROWS_PER_CORE, Q], fp32, kind="ExternalOutput"
    ).ap()
    pmem = nc.dram_tensor("pmem", [Q, C], fp32, kind="ExternalOutput").ap()

    with tile.TileContext(nc) as tc, ExitStack() as ctx:
        const = ctx.enter_context(tc.tile_pool(name="const", bufs=1))
        inpool = ctx.enter_context(tc.tile_pool(name="inblk", bufs=3))
        inr_pool = ctx.enter_context(tc.tile_pool(name="inr", bufs=4))
        outpool = ctx.enter_context(tc.tile_pool(name="outblk", bufs=3))
        ptT_pool = ctx.enter_context(tc.tile_pool(name="ptT", bufs=4))
        oh_pool = ctx.enter_context(tc.tile_pool(name="oh", bufs=6))
        rm_pool = ctx.enter_context(tc.tile_pool(name="rmax", bufs=6))
        misc = ctx.enter_context(tc.tile_pool(name="misc", bufs=1))
        ps_tr = ctx.enter_context(tc.tile_pool(name="ps_tr", bufs=3, space="PSUM"))
        ps_log = ctx.enter_context(tc.tile_pool(name="ps_log", bufs=3, space="PSUM"))
        ps_mem = ctx.enter_context(tc.tile_pool(name="ps_mem", bufs=1, space="PSUM"))

        ident = const.tile([128, 128], fp32)
        make_identity(nc, ident[:])
        # meT DRAM [256,128]; chunk k (rows k*128..) -> me_sb[:, k, :]
        me_sb = const.tile([128, 2, Q], fp32)
        nc.sync.dma_start(me_sb[:], meT.rearrange("(k c) q -> c k q", c=128))

        mem_acc = ps_mem.tile([Q, C], fp32)  # one bank, accumulates all blocks

        # per-pair state carried across the software pipeline
        inblk_of = {}   # block -> tile
        outblk_of = {}  # block -> tile
        ptT_of = {}     # pair g -> sbuf tile [128, 512]
        oh_of = {}      # pair g -> onehot tile [128, 2*Q]
        inr_of = {}     # pair g -> fp32r point rows [128, 512]
        scat_count = 0

        def stage_A(g):
            """DMA block if needed; transpose pair g's two tiles; copy to SBUF."""
            b, pair = divmod(g, TILES_PER_BLOCK // 2)
            if pair == 0:
                inblk = inpool.tile([128, TILES_PER_BLOCK * C], fp32, name="inblk")
                nc.sync.dma_start(
                    inblk[:].rearrange("p (j c) -> p j c", j=TILES_PER_BLOCK),
                    pts[b * BLOCK_ROWS : (b + 1) * BLOCK_ROWS, :].rearrange(
                        "(p j) c -> p j c", p=128
                    ),
                )
                inblk_of[b] = inblk
                outblk_of[b] = outpool.tile(
                    [128, TILES_PER_BLOCK * Q], fp32, name="outblk"
                )
            inblk = inblk_of[b]
            j0 = pair * 2
            trb = ps_tr.tile([128, 512], fp32)
            for jj in range(2):
                for k in range(2):
                    nc.tensor.transpose(
                        trb[:, (jj * 2 + k) * 128 : (jj * 2 + k + 1) * 128],
                        inblk[:, (j0 + jj) * C + k * 128 : (j0 + jj) * C + (k + 1) * 128],
                        ident[:],
                    )
            ptT = ptT_pool.tile([128, 512], fp32)
            nc.vector.tensor_copy(ptT[:, 0:256], trb[:, 0:256])
            nc.scalar.copy(ptT[:, 256:512], trb[:, 256:512])
            ptT_of[g] = ptT
            # bf16 copy of the pair's point rows for the scatter matmul
            # (runs at full PE rate; bf16 rounding of P costs ~1e-3 on the
            # cluster sums, far under tolerance; onehot is exact in bf16)
            inr = inr_pool.tile([128, 512], bf16, name="inr")
            nc.scalar.copy(inr[:], inblk[:, j0 * C : (j0 + 2) * C])
            inr_of[g] = inr

        def stage_B(g):
            """Logits matmuls for pair g; stage to outblk; rowmax; onehot."""
            b, pair = divmod(g, TILES_PER_BLOCK // 2)
            j0 = pair * 2
            ptT = ptT_of.pop(g)
            outblk = outblk_of[b]
            logb = ps_log.tile([128, 2 * Q], fp32)
            for jj in range(2):
                nc.tensor.matmul(
                    logb[:, jj * Q : (jj + 1) * Q],
                    ptT[:, jj * 256 : jj * 256 + 128],
                    me_sb[:, 0, :],
                    start=True,
                    stop=False,
                )
                nc.tensor.matmul(
                    logb[:, jj * Q : (jj + 1) * Q],
                    ptT[:, jj * 256 + 128 : jj * 256 + 256],
                    me_sb[:, 1, :],
                    start=False,
                    stop=True,
                )
            nc.scalar.copy(outblk[:, j0 * Q : (j0 + 2) * Q], logb[:])
            rmax = rm_pool.tile([128, 2], fp32)
            nc.vector.tensor_reduce(
                rmax[:],
                outblk[:, j0 * Q : (j0 + 2) * Q].rearrange("p (j q) -> p j q", j=2),
                axis=AX.X,
                op=OP.max,
            )
            oh = oh_pool.tile([128, 2 * Q], bf16)
            for jj in range(2):
                nc.vector.tensor_scalar(
                    oh[:, jj * Q : (jj + 1) * Q],
                    outblk[:, (j0 + jj) * Q : (j0 + jj + 1) * Q],
                    rmax[:, jj : jj + 1],
                    None,
                    op0=OP.is_ge,
                )
            oh_of[g] = oh

        def stage_C(g):
            """Scatter matmuls for pair g into the persistent accumulator."""
            nonlocal scat_count
            b, pair = divmod(g, TILES_PER_BLOCK // 2)
            j0 = pair * 2
            oh = oh_of.pop(g)
            inr = inr_of.pop(g)
            for jj in range(2):
                nc.tensor.matmul(
                    mem_acc[:],
                    oh[:, jj * Q : (jj + 1) * Q],
                    inr[:, jj * C : (jj + 1) * C],
                    start=(scat_count == 0),
                    stop=(scat_count == 2 * N_PAIRS - 1),
                    skip_group_check=True,
                )
                scat_count += 1

        def flush_block_out(b):
            outblk = outblk_of.pop(b)
            nc.sync.dma_start(
                logits[b * BLOCK_ROWS : (b + 1) * BLOCK_ROWS, :].rearrange(
                    "(p j) q -> p j q", p=128
                ),
                outblk[:].rearrange("p (j q) -> p j q", j=TILES_PER_BLOCK),
            )

        PAIRS_PER_BLOCK = TILES_PER_BLOCK // 2
        for g in range(N_PAIRS + 2):
            if g < N_PAIRS:
                stage_A(g)
            if 1 <= g <= N_PAIRS:
                stage_B(g - 1)
                if (g - 1) % PAIRS_PER_BLOCK == PAIRS_PER_BLOCK - 1:
                    flush_block_out((g - 1) // PAIRS_PER_BLOCK)
            if g >= 2:
                stage_C(g - 2)
            # drop inblk refs for fully-consumed blocks
            done_b = (g - 2) // PAIRS_PER_BLOCK if g >= 2 else -1
            if done_b >= 0 and (g - 2) % PAIRS_PER_BLOCK == PAIRS_PER_BLOCK - 1:
                inblk_of.pop(done_b, None)

        pm_sb = misc.tile([Q, C], fp32)
        nc.vector.tensor_copy(pm_sb[:], mem_acc[:])
        nc.sync.dma_start(pmem[:], pm_sb[:])

    nc.compile()
    _cache["nc"] = nc
    return nc


def _layer_norm(x, w, b):
    m = x.mean(axis=-1, keepdims=True, dtype=np.float32)
    v = np.mean((x - m) ** 2, axis=-1, keepdims=True, dtype=np.float32)
    return ((x - m) / np.sqrt(v + EPS) * w + b).astype(np.float32)


def _host_head(cluster_centers, ln0_w, ln0_b, W1, b1, W2, b2, W3, b3):
    x = _layer_norm(cluster_centers, ln0_w, ln0_b)
    x = np.maximum(x @ W1.T + b1, 0.0).astype(np.float32)
    x = np.maximum(x @ W2.T + b2, 0.0).astype(np.float32)
    return (x @ W3.T + b3).astype(np.float32)  # mask_embeddings [Q, C]


def _make_in_maps(point_features, me):
    meT = np.ascontiguousarray(me.T).astype(np.float32)  # [C, Q]
    padded = np.zeros((N_CORES * ROWS_PER_CORE, C), np.float32)
    padded[:N_FULL] = point_features
    shards = padded.reshape(N_CORES, ROWS_PER_CORE, C)
    return [
        {"pts": np.ascontiguousarray(shards[i]), "meT": meT} for i in range(N_CORES)
    ]


def run_device(in_maps, trace=False, tmpdir=None):
    from concourse import bass_utils

    nc = _build_module()
    return bass_utils.run_bass_kernel_spmd(
        nc,
        in_maps,
        core_ids=list(range(N_CORES)),
        trace=trace,
        tmpdir=tmpdir,
    )


def kernel(
    cluster_centers,
    point_features,
    ln0_w,
    ln0_b,
    W1,
    b1,
    W2,
    b2,
    W3,
    b3,
    lnb1_w,
    lnb1_b,
    Wb,
    lnb2_w,
    lnb2_b,
):
    cluster_centers = np.asarray(cluster_centers, np.float32)
    point_features = np.asarray(point_features, np.float32)
    args = [ln0_w, ln0_b, W1, b1, W2, b2, W3, b3]
    ln0_w, ln0_b, W1, b1, W2, b2, W3, b3 = (np.asarray(a, np.float32) for a in args)
    lnb1_w = np.asarray(lnb1_w, np.float32)
    lnb1_b = np.asarray(lnb1_b, np.float32)
    Wb = np.asarray(Wb, np.float32)
    lnb2_w = np.asarray(lnb2_w, np.float32)
    lnb2_b = np.asarray(lnb2_b, np.float32)

    me = _host_head(cluster_centers, ln0_w, ln0_b, W1, b1, W2, b2, W3, b3)
    in_maps = _make_in_maps(point_features, me)
    res = run_device(in_maps)

    pred_logits = np.concatenate(
        [res.results[i]["logits"] for i in range(N_CORES)], axis=0
    )[:N_FULL]
    cluster_memory = np.zeros((Q, C), np.float64)
    for i in range(N_CORES):
        cluster_memory += res.results[i]["pmem"].astype(np.float64)
    cluster_memory = cluster_memory.astype(np.float32)

    y = _layer_norm(cluster_memory, lnb1_w, lnb1_b)
    y = (y @ Wb.T).astype(np.float32)
    y = _layer_norm(y, lnb2_w, lnb2_b)
    return pred_logits.astype(np.float32), (cluster_centers + y).astype(np.float32)
